# revision 11
# baseline (speedup 1.0000x reference)
"""MultiHeadAttention Trainium2 kernel (v2).

Full shapes: B=4, T=2048, D=1024, H=16, HD=64.
Sharding over 8 cores: core c -> batch b=c//2, head-group g=c%2 (8 heads,
feature columns g*512:(g+1)*512 of the projection space).

v2 structure (vs v1):
  - Scores for a (pair, 2-j block) of BOTH heads land in one 4-bank PSUM
    region [128, 4, 512]; ONE FD=2048 exp ACTIVATE per block (ScalarE
    ~237us instead of ~285us).
  - Softmax denominators: 4 direct ones-matmuls (j0..3) + DVE bf16
    pair-adds (j4..15) + 2 accumulator ones-matmuls per head.  Head A/B
    den rows packed in ONE PSUM bank (rows 0 / 32, col strips 0 / 1,
    concurrent).  Removes ~100us of PE ones-matmul streaming.
  - reciprocal_approx_fast instead of iterative reciprocal (saves ~100us
    DVE).
  - Q-projection of chunk c+1 and out-projection of chunk c are emitted
    inside the attention Scalar-shadow (PE filler) so the PE never idles
    long enough for HAM to re-throttle.
  - Output DMA'd as bf16 outT partials; host sums/upcasts/transposes.
"""

import os

import numpy as np

B, T, D, H = 4, 2048, 1024, 16
HD = 64
NCORES = 8
F = 512          # per-core projection features (8 heads * 64)
P = 128          # partitions
KT = D // P      # 8 k-tiles over D
MT = F // P      # 4 m-tiles over F (also head-pairs)
NCH = 4          # token chunks
CH = T // NCH    # 512 tokens per chunk
TKT = T // P     # 16 tk tiles
NBLK = TKT // 2  # 8 score blocks per pair (2 j each)
NDIRECT = 4      # j tiles whose den goes via direct ones-matmul

_CACHE = {}


def _build(with_bias: bool):
    import concourse.bass as bass
    import concourse.tile as tile
    from concourse import bacc, mybir
    from concourse.bass import ts

    f32 = mybir.dt.float32
    bf16 = mybir.dt.bfloat16

    nc = bacc.Bacc("TRN2", target_bir_lowering=False, debug=False)

    qT = nc.dram_tensor("qT", [D, T], bf16, kind="ExternalInput")
    kT = nc.dram_tensor("kT", [D, T], bf16, kind="ExternalInput")
    vT = nc.dram_tensor("vT", [D, T], bf16, kind="ExternalInput")
    wqT = nc.dram_tensor("wqT", [D, F], bf16, kind="ExternalInput")
    wkT = nc.dram_tensor("wkT", [D, F], bf16, kind="ExternalInput")
    wvT = nc.dram_tensor("wvT", [D, F], bf16, kind="ExternalInput")
    woT = nc.dram_tensor("woT", [F, D], bf16, kind="ExternalInput")
    if with_bias:
        bqs = nc.dram_tensor("bqs", [F], f32, kind="ExternalInput")
        bks = nc.dram_tensor("bks", [F], f32, kind="ExternalInput")
        bvs = nc.dram_tensor("bvs", [F], f32, kind="ExternalInput")
        bos = nc.dram_tensor("bos", [D], f32, kind="ExternalInput")
    outT = nc.dram_tensor("outT", [D, T], bf16, kind="ExternalOutput")

    with tile.TileContext(nc) as tc:
        from contextlib import ExitStack

        with ExitStack() as ctx:
            psum = ctx.enter_context(tc.tile_pool(name="ps", bufs=1, space="PSUM"))
            const = ctx.enter_context(tc.tile_pool(name="const", bufs=1))
            persist = ctx.enter_context(tc.tile_pool(name="persist", bufs=1))
            work = ctx.enter_context(tc.tile_pool(name="work", bufs=1))

            # ---- constants ----
            wk_sb = const.tile([P, KT, F], bf16, name="wk_sb")
            nc.sync.dma_start(out=wk_sb, in_=wkT[:].rearrange("(ko ki) f -> ki ko f", ki=P))
            wq_sb = const.tile([P, KT, F], bf16, name="wq_sb")
            nc.sync.dma_start(out=wq_sb, in_=wqT[:].rearrange("(ko ki) f -> ki ko f", ki=P))
            wv_sb = const.tile([P, KT, F], bf16, name="wv_sb")
            nc.sync.dma_start(out=wv_sb, in_=wvT[:].rearrange("(ko ki) f -> ki ko f", ki=P))
            wo_sb = const.tile([P, MT, D], bf16, name="wo_sb")
            nc.sync.dma_start(out=wo_sb, in_=woT[:].rearrange("(ko ki) f -> ki ko f", ki=P))
            ones_pv = const.tile([P, 1], bf16, name="ones_pv")
            nc.vector.memset(ones_pv, 1.0)

            if with_bias:
                bq_sb = const.tile([P, MT], f32, name="bq_sb")
                nc.sync.dma_start(out=bq_sb, in_=bqs[:].rearrange("(m p) -> p m", p=P))
                bk_sb = const.tile([P, MT], f32, name="bk_sb")
                nc.sync.dma_start(out=bk_sb, in_=bks[:].rearrange("(m p) -> p m", p=P))
                bo_sb = const.tile([P, D // P], f32, name="bo_sb")
                nc.sync.dma_start(out=bo_sb, in_=bos[:].rearrange("(m p) -> p m", p=P))
                bv_bc = const.tile([P, F], f32, name="bv_bc")
                bvs_ap = bvs[:]
                nc.sync.dma_start(
                    out=bv_bc,
                    in_=bass.AP(
                        tensor=bvs_ap.tensor, offset=bvs_ap.offset,
                        ap=[[0, P], *bvs_ap.ap],
                    ),
                )

            # ---- persistent activations ----
            qpt = persist.tile([P, MT, T], bf16, name="qpt")   # qp^T [feat, tok]
            kpt = persist.tile([P, MT, T], bf16, name="kpt")   # kp^T
            vp = persist.tile([P, TKT, F], bf16, name="vp")    # vp   [tok, feat]

            # ============ K / V projections (head phase) ============
            def load_raw(src, c, tag):
                t = work.tile([P, KT, CH], bf16, name=tag, tag=tag, bufs=2)
                nc.sync.dma_start(
                    out=t,
                    in_=src[:].rearrange("(ko ki) t -> ki ko t", ki=P)[:, :, ts(c, CH)],
                )
                return t

            for c in range(NCH):
                k_raw = load_raw(kT, c, "kv_raw")
                cs = ts(c, CH)
                for half in range(2):
                    blk = psum.tile([P, 2, CH], f32, name="kproj", tag="blk", bufs=2)
                    for mm_ in range(2):
                        m = 2 * half + mm_
                        for k in range(KT):
                            nc.tensor.matmul(
                                blk[:, mm_, :], lhsT=wk_sb[:, k, ts(m, P)], rhs=k_raw[:, k, :],
                                start=(k == 0), stop=(k == KT - 1),
                            )
                    if with_bias:
                        for mm_ in range(2):
                            m = 2 * half + mm_
                            nc.vector.tensor_scalar_add(
                                out=kpt[:, m, cs], in0=blk[:, mm_, :], scalar1=bk_sb[:, m : m + 1]
                            )
                    else:
                        nc.vector.tensor_copy(out=kpt[:, 2 * half : 2 * half + 2, cs], in_=blk)

            def vproj_half(c, half, cell):
                def emit():
                    if half == 0:
                        cell["raw"] = load_raw(vT, c, "kv_raw")
                    v_raw = cell["raw"]
                    blk = psum.tile([P, 2, CH], f32, name="vproj", tag="blk", bufs=2)
                    for tt_ in range(2):
                        tt = 2 * half + tt_
                        for k in range(KT):
                            nc.tensor.matmul(
                                blk[:, tt_, :], lhsT=v_raw[:, k, ts(tt, P)], rhs=wv_sb[:, k, :],
                                start=(k == 0), stop=(k == KT - 1),
                            )
                    base = c * MT + 2 * half
                    nc.vector.tensor_copy(out=vp[:, base : base + 2, :], in_=blk)
                    if with_bias:
                        for tt_ in range(2):
                            nc.vector.tensor_add(
                                out=vp[:, base + tt_, :], in0=vp[:, base + tt_, :],
                                in1=bv_bc,
                            )
                return emit

            # ============ chunk loop ============
            def qproj_half(c, half, cell):
                def emit():
                    if half == 0:
                        cell["raw"] = load_raw(qT, c, "q_raw")
                    q_raw = cell["raw"]
                    cs = ts(c, CH)
                    blk = psum.tile([P, 2, CH], f32, name="qproj", tag="blk", bufs=2)
                    for mm_ in range(2):
                        m = 2 * half + mm_
                        for k in range(KT):
                            nc.tensor.matmul(
                                blk[:, mm_, :], lhsT=wq_sb[:, k, ts(m, P)], rhs=q_raw[:, k, :],
                                start=(k == 0), stop=(k == KT - 1),
                            )
                    if with_bias:
                        for mm_ in range(2):
                            m = 2 * half + mm_
                            nc.vector.tensor_scalar_add(
                                out=qpt[:, m, cs], in0=blk[:, mm_, :], scalar1=bq_sb[:, m : m + 1]
                            )
                    else:
                        nc.vector.tensor_copy(out=qpt[:, 2 * half : 2 * half + 2, cs], in_=blk)
                return emit

            def emit_qproj(c):
                cell = {}
                for half in range(2):
                    qproj_half(c, half, cell)()

            def outproj_group(c, m, aot):
                def emit():
                    pot = psum.tile([P, 2, CH], f32, name="po", tag="blk", bufs=2)
                    po = pot[:, 0, :]
                    for pp in range(MT):
                        nc.tensor.matmul(
                            po, lhsT=wo_sb[:, pp, ts(m, P)], rhs=aot[:, pp, :],
                            start=(pp == 0), stop=(pp == MT - 1),
                        )
                    ot = work.tile([P, CH], bf16, name="ot", tag="ot", bufs=3)
                    if with_bias:
                        nc.vector.tensor_scalar_add(
                            out=ot, in0=po, scalar1=bo_sb[:, m : m + 1]
                        )
                    else:
                        nc.vector.tensor_copy(out=ot, in_=po)
                    nc.sync.dma_start(out=outT[ts(m, P), ts(c, CH)], in_=ot)
                return emit

            filler = []  # deferred PE work (outproj groups of previous chunk)

            emit_qproj(0)
            for c in range(NCH):
                vcell = {}
                if c == 0:
                    for vc in range(NCH):
                        cell = {}
                        filler.append(vproj_half(vc, 0, cell))
                        filler.append(vproj_half(vc, 1, cell))
                cs = ts(c, CH)
                aot = work.tile([P, MT, CH], bf16, name="aot", tag="aot", bufs=2)
                for p in range(MT):  # head pair (2p, 2p+1)
                    if p == MT - 1 and c + 1 < NCH:
                        qcell = {}
                        filler.append(qproj_half(c + 1, 0, qcell))
                        filler.append(qproj_half(c + 1, 1, qcell))
                    scores_first = (c == 0 and p == 0)
                    exp_t = work.tile([P, 2, TKT, CH], bf16, name="exp_t", tag="exp", bufs=2)
                    pvp = psum.tile([P, CH], f32, name="pvp", tag="acc", bufs=2)
                    dent = psum.tile([P, CH], f32, name="dent", tag="den", bufs=1)
                    dent2 = psum.tile([P, CH], f32, name="dent2", tag="den2", bufs=1)
                    accA = work.tile([P, 2, CH], bf16, name="accA", tag="accA", bufs=1)
                    accB = work.tile([P, 2, CH], bf16, name="accB", tag="accB", bufs=1)

                    def pv_mm(j):
                        st, sp = (j == 0), (j == TKT - 1)
                        nc.tensor.matmul(
                            pvp[0:HD, :], lhsT=vp[:, j, ts(2 * p, HD)],
                            rhs=exp_t[:, 0, j, :], start=st, stop=sp,
                            skip_group_check=True, tile_position=(0, 0),
                        )
                        nc.tensor.matmul(
                            pvp[HD:P, :], lhsT=vp[:, j, ts(2 * p + 1, HD)],
                            rhs=exp_t[:, 1, j, :], start=st, stop=sp,
                            skip_group_check=True, tile_position=(0, HD),
                        )

                    def den_direct(j, first):
                        nc.tensor.matmul(
                            dent[0:1, :], lhsT=ones_pv, rhs=exp_t[:, 0, j, :],
                            start=first, stop=False,
                            skip_group_check=True, tile_position=(0, 0),
                        )
                        nc.tensor.matmul(
                            dent2[0:1, :], lhsT=ones_pv, rhs=exp_t[:, 1, j, :],
                            start=first, stop=False,
                            skip_group_check=True, tile_position=(0, 0),
                        )

                    for b in range(NBLK):
                        tA = psum.tile([P, 2, CH], f32, name="qkA", tag="blk", bufs=2)
                        tB = psum.tile([P, 2, CH], f32, name="qkB", tag="blk", bufs=2)
                        for u in range(2):
                            j = 2 * b + u
                            nc.tensor.matmul(
                                tA[:, u, :],
                                lhsT=kpt[0:HD, p, ts(j, P)], rhs=qpt[0:HD, p, cs],
                            )
                            nc.tensor.matmul(
                                tB[:, u, :],
                                lhsT=kpt[HD:P, p, ts(j, P)], rhs=qpt[HD:P, p, cs],
                            )
                        nc.scalar.activation(
                            out=exp_t[:, 0, 2 * b : 2 * b + 2, :], in_=tA,
                            func=mybir.ActivationFunctionType.Exp, scale=0.125,
                        )
                        nc.scalar.activation(
                            out=exp_t[:, 1, 2 * b : 2 * b + 2, :], in_=tB,
                            func=mybir.ActivationFunctionType.Exp, scale=0.125,
                        )
                        if b >= 1 and not scores_first:
                            pv_mm(2 * (b - 1))
                            pv_mm(2 * (b - 1) + 1)
                        # DVE accumulation of all j (bf16 pair adds)
                        if b == 2:
                            nc.vector.tensor_add(
                                out=accA, in0=exp_t[:, 0, 0:2, :], in1=exp_t[:, 0, 2:4, :]
                            )
                            nc.vector.tensor_add(
                                out=accB, in0=exp_t[:, 1, 0:2, :], in1=exp_t[:, 1, 2:4, :]
                            )
                        elif b in (3, 4, 5, 6, 7):
                            lo = 2 * b - 2  # j pair produced by block b-1
                            nc.vector.tensor_add(
                                out=accA, in0=accA, in1=exp_t[:, 0, lo : lo + 2, :]
                            )
                            nc.vector.tensor_add(
                                out=accB, in0=accB, in1=exp_t[:, 1, lo : lo + 2, :]
                            )
                        if filler and (scores_first or b % 2 == 1):
                            filler.pop(0)()
                    # tail: last PV pair, last den adds, den accumulation MMs
                    if scores_first:
                        for j in range(TKT - 2):
                            pv_mm(j)
                    pv_mm(TKT - 2)
                    pv_mm(TKT - 1)
                    nc.vector.tensor_add(
                        out=accA, in0=accA, in1=exp_t[:, 0, 14:16, :]
                    )
                    nc.vector.tensor_add(
                        out=accB, in0=accB, in1=exp_t[:, 1, 14:16, :]
                    )
                    for a in range(2):
                        nc.tensor.matmul(
                            dent[0:1, :], lhsT=ones_pv, rhs=accA[:, a, :],
                            start=(a == 0), stop=(a == 1),
                            skip_group_check=True, tile_position=(0, 0),
                        )
                        nc.tensor.matmul(
                            dent2[0:1, :], lhsT=ones_pv, rhs=accB[:, a, :],
                            start=(a == 0), stop=(a == 1),
                            skip_group_check=True, tile_position=(0, 0),
                        )
                    recA = work.tile([1, CH], f32, name="recA", tag="recA", bufs=2)
                    recB = work.tile([1, CH], f32, name="recB", tag="recB", bufs=2)
                    nc.vector.reciprocal_approx_fast(out=recA, in_=dent[0:1, :])
                    nc.vector.reciprocal_approx_fast(out=recB, in_=dent2[0:1, :])
                    bc1 = work.tile([P, CH], f32, name="bc1", tag="bc1", bufs=1)
                    bc2 = work.tile([P, CH], f32, name="bc2", tag="bc2", bufs=1)
                    nc.gpsimd.partition_broadcast(bc1[:, :], recA[0:1, :])
                    nc.gpsimd.partition_broadcast(bc2[:, :], recB[0:1, :])
                    nc.vector.tensor_mul(
                        out=aot[0:HD, p, :], in0=pvp[0:HD, :], in1=bc1[0:HD, :]
                    )
                    nc.vector.tensor_mul(
                        out=aot[HD:P, p, :], in0=pvp[HD:P, :], in1=bc2[HD:P, :]
                    )
                while filler:
                    filler.pop(0)()
                # out-projection of this chunk becomes filler for the next
                # chunk's early blocks (last chunk: emit directly)
                groups = [outproj_group(c, m, aot) for m in range(D // P)]
                if c + 1 < NCH:
                    filler.extend(groups)
                else:
                    for g in groups:
                        g()
    nc.compile()
    return nc


def kernel(q, k, v, Wq, bq, Wk, bk, Wv, bv, Wo, bo):
    from concourse.bass_utils import run_bass_kernel_spmd

    q, k, v = (np.asarray(x, np.float32) for x in (q, k, v))
    Wq, Wk, Wv, Wo = (np.asarray(x, np.float32) for x in (Wq, Wk, Wv, Wo))
    bq, bk, bv, bo = (np.asarray(x, np.float32) for x in (bq, bk, bv, bo))

    with_bias = bool(
        np.any(bq) or np.any(bk) or np.any(bv) or np.any(bo)
    )
    key = ("nc", with_bias)
    if key not in _CACHE:
        _CACHE[key] = _build(with_bias)
    nc = _CACHE[key]
    _CACHE["nc"] = nc

    import ml_dtypes

    bf = ml_dtypes.bfloat16
    in_maps = []
    for c in range(NCORES):
        b, g = c // 2, c % 2
        cols = slice(g * F, (g + 1) * F)
        im = {
            "qT": np.ascontiguousarray(q[b].T).astype(bf),
            "kT": np.ascontiguousarray(k[b].T).astype(bf),
            "vT": np.ascontiguousarray(v[b].T).astype(bf),
            "wqT": np.ascontiguousarray(Wq[cols, :].T).astype(bf),
            "wkT": np.ascontiguousarray(Wk[cols, :].T).astype(bf),
            "wvT": np.ascontiguousarray(Wv[cols, :].T).astype(bf),
            "woT": np.ascontiguousarray(Wo[:, cols].T).astype(bf),
        }
        if with_bias:
            im.update({
                "bqs": np.ascontiguousarray(bq[cols]),
                "bks": np.ascontiguousarray(bk[cols]),
                "bvs": np.ascontiguousarray(bv[cols]),
                # bo applied once per batch (head-group 0 only)
                "bos": np.ascontiguousarray(bo if g == 0 else np.zeros_like(bo)),
            })
        in_maps.append(im)

    _CACHE["in_maps"] = in_maps
    trace = bool(int(os.environ.get("KERNEL_TRACE", "0")))
    res = run_bass_kernel_spmd(
        nc, in_maps, core_ids=list(range(NCORES)), trace=trace
    )
    if trace and res.exec_time_ns is not None:
        print(f"HW exec time: {res.exec_time_ns} ns")
    outs = [r["outT"].astype(np.float32) for r in res.results]
    out = np.empty((B, T, D), np.float32)
    for b in range(B):
        out[b] = (outs[2 * b] + outs[2 * b + 1]).T
    return out
